# revision 21
# baseline (speedup 1.0000x reference)
"""Trainium2 Bass kernel for nn_ArmModel (7-DOF arm RK4 dynamics step + FK).

Self-contained: hardcodes shapes/sharding. 8-core pure data parallelism over
the batch (16384 -> 2048/core). Per core, batch is laid out as 128 partitions
x 16 free columns; every scalar quantity ("channel") is a (128,16) block at a
free-dim offset of one big SBUF tile. All math is emitted as vector/scalar
engine ops with strided/broadcast access patterns.

Algorithm (mathematically identical to the reference, restructured):
  T_i(q)   = exp(-q_i [A_i]) @ Minv_i gives Ad = [[R,0],[Q,R]], Q = hat(p) R
  R        = K0 - s*K1 + (1-c)*K2           (K* host-precomputed per joint)
  p        = e0 + q*e1 + s*e2 + (1-c)*e3
  forward:  bundle [V, Vd, U_j] propagated by Ad per joint;
            Vd += -ad(A_i) V_i * dq_i ; V += A_i dq_i
  bias_i   = sum_{k>=i} U_{k,i} . Y_k,  Y_k = G Vd_k - ad^T(V_k) G V_k
  M_{ij}   = sum_{k>=max(i,j)} U_{k,i} . (G_k U_{k,j})   (Gram form)
  solve    = LDL^T (no pivoting; M SPD)
  RK4, wrap q, clip dq, FK rows-accumulation for end-effector (x,y).
"""
import numpy as np

DOF = 7
BATCH = 16384
NCORES = 8
PB = BATCH // NCORES      # 2048 per core
P = 128
T = 16                    # batch cols per channel; PB = P*T
H = 0.1
ACTION_RANGE = 50.0
MAX_VELOCITY = 20.0

F32 = None  # set lazily (mybir.dt.float32)


# ============================ constant layout ============================
class ConstLayout:
    def __init__(self):
        self.off = {}
        self.n = 0

    def alloc(self, name, n):
        self.off[name] = self.n
        self.n += n
        return self.off[name]


CL = ConstLayout()
CL.alloc("A", 42)          # [j*6+c]
CL.alloc("nadA", 252)      # [j*36 + r*6 + c]   (-ad(A_j))
CL.alloc("g", 42)          # [k*6+c]
CL.alloc("gRr1", 21)       # [k*3+c] = gI[p1(c)]
CL.alloc("gRr2", 21)
CL.alloc("gm3", 21)        # [k*3+c] = mass scalar replicated
CL.alloc("K0", 63)         # [j*9 + r*3 + c]
CL.alloc("K1n", 63)        # -K1
CL.alloc("K2", 63)
CL.alloc("e0", 21)
CL.alloc("e1", 21)
CL.alloc("e2", 21)
CL.alloc("e3", 21)
CL.alloc("F0", 63)
CL.alloc("F1", 63)
CL.alloc("F2", 63)
CL.alloc("f0", 21)
CL.alloc("f1", 21)
CL.alloc("f2", 21)
CL.alloc("f3", 21)
CL.alloc("pe", 3)
CL.alloc("vd0", 6)         # (0,0,0,-gravity)
CL.alloc("r01i", 6)        # rows init (1,0,0),(0,1,0)
CL.alloc("w4", 4)          # RK weights 1,2,2,1
CL.alloc("halfpi", 1)
CL.alloc("gA1", 42)        # [gRr1 | gm3]
CL.alloc("gA2", 42)        # [gRr2 | gm3]
NCOL = CL.n


def _hat(w):
    return np.array([[0, -w[2], w[1]], [w[2], 0, -w[0]], [-w[1], w[0], 0]],
                    np.float64)


def make_consts(_M, _A, _G, gravity):
    """Host-side: build the (128, NCOL) f32 constant tile."""
    _M = np.asarray(_M, np.float64)
    _A = np.asarray(_A, np.float64)
    _G = np.asarray(_G, np.float64)
    gravity = np.asarray(gravity, np.float64)
    buf = np.zeros((NCOL,), np.float64)
    o = CL.off
    p1 = [1, 2, 0]
    p2 = [2, 0, 1]

    Ms = []
    for k in range(DOF + 1):
        a1, a2, p = _M[k, :3, 0], _M[k, :3, 1], _M[k, :3, 3]
        b1 = a1 / np.linalg.norm(a1)
        a2o = a2 - (a2 @ b1) * b1
        b2 = a2o / np.linalg.norm(a2o)
        b3 = np.cross(b1, b2)
        R = np.stack([b1, b2, b3], -1)
        Tm = np.eye(4)
        Tm[:3, :3] = R
        Tm[:3, 3] = p
        Ms.append(Tm)

    gabs = np.abs(_G)
    for i in range(DOF):
        w = _A[i, :3] / np.linalg.norm(_A[i, :3])
        v = _A[i, 3:]
        A6 = np.concatenate([w, v])
        W = _hat(w)
        W2 = W @ W
        R_, p_ = Ms[i][:3, :3], Ms[i][:3, 3]
        Rm, pm = R_.T, -R_.T @ p_

        buf[o["A"] + 6 * i: o["A"] + 6 * i + 6] = A6
        # ad(A) = [[hat(w),0],[hat(v),hat(w)]]; store negated
        adA = np.zeros((6, 6))
        adA[:3, :3] = W
        adA[3:, 3:] = W
        adA[3:, :3] = _hat(v)
        buf[o["nadA"] + 36 * i: o["nadA"] + 36 * (i + 1)] = (-adA).ravel()

        gd = np.concatenate([gabs[i, :3], np.repeat(gabs[i, 3], 3)])
        buf[o["g"] + 6 * i: o["g"] + 6 * i + 6] = gd
        buf[o["gRr1"] + 3 * i: o["gRr1"] + 3 * i + 3] = gd[:3][p1]
        buf[o["gRr2"] + 3 * i: o["gRr2"] + 3 * i + 3] = gd[:3][p2]
        buf[o["gm3"] + 3 * i: o["gm3"] + 3 * i + 3] = gd[3]

        buf[o["K0"] + 9 * i: o["K0"] + 9 * (i + 1)] = Rm.ravel()
        buf[o["K1n"] + 9 * i: o["K1n"] + 9 * (i + 1)] = (-(W @ Rm)).ravel()
        buf[o["K2"] + 9 * i: o["K2"] + 9 * (i + 1)] = (W2 @ Rm).ravel()
        buf[o["e0"] + 3 * i: o["e0"] + 3 * i + 3] = pm
        buf[o["e1"] + 3 * i: o["e1"] + 3 * i + 3] = -(v + W2 @ v)
        buf[o["e2"] + 3 * i: o["e2"] + 3 * i + 3] = W2 @ v - W @ pm
        buf[o["e3"] + 3 * i: o["e3"] + 3 * i + 3] = W2 @ pm + W @ v

        RM, pM = Ms[i][:3, :3], Ms[i][:3, 3]
        buf[o["F0"] + 9 * i: o["F0"] + 9 * (i + 1)] = RM.ravel()
        buf[o["F1"] + 9 * i: o["F1"] + 9 * (i + 1)] = (RM @ W).ravel()
        buf[o["F2"] + 9 * i: o["F2"] + 9 * (i + 1)] = (RM @ W2).ravel()
        buf[o["f0"] + 3 * i: o["f0"] + 3 * i + 3] = pM
        buf[o["f1"] + 3 * i: o["f1"] + 3 * i + 3] = RM @ (v + W2 @ v)
        buf[o["f2"] + 3 * i: o["f2"] + 3 * i + 3] = -RM @ (W2 @ v)
        buf[o["f3"] + 3 * i: o["f3"] + 3 * i + 3] = RM @ (W @ v)

    buf[o["pe"]: o["pe"] + 3] = Ms[DOF][:3, 3]
    buf[o["vd0"] + 3: o["vd0"] + 6] = -gravity
    buf[o["r01i"]: o["r01i"] + 6] = [1, 0, 0, 0, 1, 0]
    buf[o["w4"]: o["w4"] + 4] = [1, 2, 2, 1]
    buf[o["halfpi"]] = np.pi / 2
    buf[o["gA1"]: o["gA1"] + 21] = buf[o["gRr1"]: o["gRr1"] + 21]
    buf[o["gA1"] + 21: o["gA1"] + 42] = buf[o["gm3"]: o["gm3"] + 21]
    buf[o["gA2"]: o["gA2"] + 21] = buf[o["gRr2"]: o["gRr2"] + 21]
    buf[o["gA2"] + 21: o["gA2"] + 42] = buf[o["gm3"]: o["gm3"] + 21]
    return np.tile(buf.astype(np.float32)[None, :], (P, 1))


# ============================ channel layout ============================
class ChLayout:
    def __init__(self):
        self.off = {}
        self.n = 0

    def alloc(self, name, n):
        self.off[name] = self.n
        self.n += n
        return self.off[name]


CH = ChLayout()
CH.alloc("stag_in", 14)     # state staging [t][c] interleaved
CH.alloc("stag_act", 7)
CH.alloc("q0", 7)
CH.alloc("dq0", 7)          # == dqs block s=0 (must follow q0)
CH.alloc("dqs1", 7)
CH.alloc("dqs2", 7)
CH.alloc("dqs3", 7)
CH.alloc("qacc", 28)        # 4 stages x 7 (used as rhs/z during solve)
CH.alloc("tau", 7)
CH.alloc("qs", 7)
CH.alloc("sarg", 7)
CH.alloc("carg", 7)
CH.alloc("sn", 7)
CH.alloc("cs", 7)
CH.alloc("oc", 7)
CH.alloc("Mf", 126)         # [j*18 + r*6 + c6]; c6 0..2 = Q, 3..5 = R
CH.alloc("pT", 21)
CH.alloc("pr1", 21)
CH.alloc("pr2", 21)
CH.alloc("tb1", 63)
CH.alloc("tqp", 63)
CH.alloc("tqn", 63)
CH.alloc("rows", 12 + 7 * 54)   # init [V,Vd] then per joint [V,Vd,U_0..U_6]
CH.alloc("Ut", 294)         # [k*42 + j*6 + c]
CH.alloc("wr1", 21)
CH.alloc("vr1", 21)
CH.alloc("wr2", 21)
CH.alloc("vr2", 21)
CH.alloc("gwv1", 42)
CH.alloc("gwv2", 42)
CH.alloc("tcp", 84)
CH.alloc("tcn", 84)
CH.alloc("tcs", 84)
CH.alloc("adv12", 84)
CH.alloc("Y", 42)
CH.alloc("pos1", 21)
CH.alloc("neg1", 21)
CH.alloc("pos2", 21)
CH.alloc("neg2", 21)
CH.alloc("pos3", 21)
CH.alloc("neg3", 21)
CH.alloc("tc1", 21)
CH.alloc("bias", 7)
CH.alloc("Mg", 49)          # mass matrix grid [i*7+j] (lower valid)
CH.alloc("Lg", 49)
CH.alloc("rinv", 7)
CH.alloc("tpL", 144)
CH.alloc("tpU", 72)
CH.alloc("tadv", 36)
CH.alloc("adv", 6)
CH.alloc("advb", 6)
CH.alloc("tv6", 12)
CH.alloc("tgram", 96)
CH.alloc("tg2", 16)
CH.alloc("wy", 7)
CH.alloc("wt", 7)
CH.alloc("wm", 7)
CH.alloc("tbias", 42)
CH.alloc("tso", 36)
CH.alloc("tss", 7)
CH.alloc("tw", 28)
CH.alloc("tk7", 7)
CH.alloc("qn", 7)
CH.alloc("dqn", 7)
CH.alloc("outch", 16)       # q(7) dq(7) px py
CH.alloc("RB", 63)
CH.alloc("pB", 21)
CH.alloc("s2", 7)
CH.alloc("c2", 7)
CH.alloc("oc2", 7)
CH.alloc("ca2", 7)
CH.alloc("oc2t", 7)
CH.alloc("rcur", 6)
CH.alloc("rnxt", 6)
CH.alloc("tfr", 18)
CH.alloc("tfd", 6)
CH.alloc("tpd", 2)
CH.alloc("stag_out", 16)
NCH = CH.n


# ============================ emit ============================
def emit(nc, tc, pool, state_d, action_d, consts_d, out_d):
    import concourse.bass as bass
    from concourse import mybir

    f32 = mybir.dt.float32
    AO = mybir.AluOpType
    ACT = mybir.ActivationFunctionType
    S = pool.tile([P, NCH * T], f32)
    CT = pool.tile([P, NCOL], f32)
    SI = pool.tile([P, 7 * T], mybir.dt.int32)
    o = CH.off
    co = CL.off

    def _merge(free):
        out = []
        for st, n in free:
            if out and out[-1][0] == st * n:
                out[-1] = [st, n * out[-1][1]]
            else:
                out.append([st, n])
        assert len(out) <= 3, f"AP has {len(out)} free dims after merge: {out}"
        return out

    def dap(ch, dims=(), bpos=None):
        """Data AP. dims: list of (step_in_channels, count). Batch dim [1,T]
        appended last unless bpos gives its index within dims. Adjacent
        contiguous dims are merged (HW allows max 3 free dims)."""
        free = [[st * T, n] for st, n in dims]
        if bpos is None:
            free = free + [[1, T]]
        else:
            free = free[:bpos] + [[1, T]] + free[bpos:]
        return bass.AP(S.tensor, S.offset + ch * T,
                       [list(S.ap[0])] + _merge(free))

    def cap(col, dims=(), bpos=None):
        free = [[st, n] for st, n in dims]
        if bpos is None:
            free = free + [[0, T]]
        else:
            free = free[:bpos] + [[0, T]] + free[bpos:]
        return bass.AP(CT.tensor, CT.offset + col,
                       [list(CT.ap[0])] + _merge(free))

    V = nc.vector
    SC = nc.scalar
    G = nc.gpsimd

    def tt(out, a, b, op=AO.mult, eng=V):
        eng.tensor_tensor(out, a, b, op)

    def ts(out, a, s1, s2=None, op0=AO.mult, op1=AO.add, eng=V):
        if s2 is None:
            eng.tensor_scalar(out, a, s1, None, op0)
        else:
            eng.tensor_scalar(out, a, s1, s2, op0, op1)

    def red(out, a, axis="X", op=AO.add):
        V.tensor_reduce(out, a, getattr(mybir.AxisListType, axis), op)

    def cp(out, a, eng=None):
        V.tensor_copy(out, a)

    PI = float(np.pi)
    TWO_PI = float(2 * np.pi)

    halfpi_ap = bass.AP(CT.tensor, CT.offset + co["halfpi"],
                        [list(CT.ap[0]), [0, 1]])

    def wrap_to(out_ch, x_ch, n=7):
        """out = x - 2pi*round(x/2pi), in [-pi, pi]. The HW f32->i32 cast
        rounds to nearest-even; floor((x+pi)/2pi) == round(x/2pi) away from
        ties, so this matches the reference mod semantics."""
        ts(dap(o["wy"], [(1, n)]), dap(x_ch, [(1, n)]), 1.0 / TWO_PI, None)
        V.tensor_copy(SI[:, :n * T], dap(o["wy"], [(1, n)]))
        V.tensor_copy(dap(o["wt"], [(1, n)]), SI[:, :n * T])
        V.scalar_tensor_tensor(dap(out_ch, [(1, n)]), dap(o["wt"], [(1, n)]),
                               -TWO_PI, dap(x_ch, [(1, n)]), AO.mult, AO.add)

    def sincos(s_ch, c_ch, arg_ch, n=7):
        """s = sin(arg), c = cos(arg) for arg in [-pi, pi]. cos via
        sin(pi/2 - |arg|) so the ACT input stays in range."""
        SC.activation(dap(s_ch, [(1, n)]), dap(arg_ch, [(1, n)]), ACT.Sin)
        SC.activation(dap(o["wm"], [(1, n)]), dap(arg_ch, [(1, n)]), ACT.Abs)
        SC.activation(dap(c_ch, [(1, n)]), dap(o["wm"], [(1, n)]), ACT.Sin,
                      bias=halfpi_ap, scale=-1.0)

    # ---------- load inputs ----------
    nc.sync.dma_start(S[:, o["stag_in"] * T:(o["stag_in"] + 14) * T],
                      state_d[:].rearrange("(p t) c -> p (t c)", p=P))
    nc.sync.dma_start(S[:, o["stag_act"] * T:(o["stag_act"] + 7) * T],
                      action_d[:].rearrange("(p t) c -> p (t c)", p=P))
    nc.sync.dma_start(CT[:], consts_d[:])

    # de-interleave: [t][c] -> channel-major q0(7),dq0(7)
    cp(dap(o["q0"], [(1, 14)], bpos=1),
       bass.AP(S.tensor, S.offset + o["stag_in"] * T,
               [list(S.ap[0])] + [[1, 14], [14, T]]))
    # tau = action * 50
    ts(dap(o["tau"], [(1, 7)], bpos=1),
       bass.AP(S.tensor, S.offset + o["stag_act"] * T,
               [list(S.ap[0])] + [[1, 7], [7, T]]),
       ACTION_RANGE, None)

    # init row: V=0, Vd=(0,0,0,-gravity)
    V.memset(dap(o["rows"], [(1, 6)]), 0.0)
    cp(dap(o["rows"] + 6, [(1, 6)]), cap(co["vd0"], [(1, 6)]))
    # memset M grid upper-garbage guard + U grid (avoid NaN reads in sim)
    V.memset(dap(o["Mg"], [(1, 49)]), 0.0)

    # adv12 blocks: [A_j (6) | adv_j (6)] per joint; A halves are static
    cp(bass.AP(S.tensor, S.offset + o["adv12"] * T,
               [list(S.ap[0]), [12 * T, 7], [1, 6 * T]]),
       cap(co["A"], [(6, 7), (1, 6)]), eng=V)

    dqs_blocks = [o["dq0"], o["dqs1"], o["dqs2"], o["dqs3"]]
    qs_ch = [o["q0"], o["qsB"], o["qs"], o["qsB"]]

    def C(name, stage):
        return o[name + "B"] if (stage % 2) else o[name]

    def prep_qs(stage):
        # qs_s = q0 + a_h*dqs_{s-1}; depends on solve_{s-2} only
        a_h = [None, 0.5 * H, 0.5 * H, H][stage]
        V.scalar_tensor_tensor(dap(qs_ch[stage], [(1, 7)]),
            dap(dqs_blocks[stage - 1], [(1, 7)]), a_h,
            dap(o["q0"], [(1, 7)]), AO.mult, AO.add)

    def prep_dqs(stage):
        a_h = [None, 0.5 * H, 0.5 * H, H][stage]
        V.scalar_tensor_tensor(dap(dqs_blocks[stage], [(1, 7)]),
            dap(o["qacc"] + 7 * (stage - 1), [(1, 7)]), a_h,
            dap(o["dq0"], [(1, 7)]), AO.mult, AO.add)

    def emit_build(stage):
        qs = qs_ch[stage]
        sarg, sn, cs, oc = (C("sarg", stage), C("sn", stage), C("cs", stage),
                            C("oc", stage))
        tb1, Mf, pT = C("tb1", stage), C("Mf", stage), C("pT", stage)
        pr1, pr2, tqp, tqn = (C("pr1", stage), C("pr2", stage),
                              C("tqp", stage), C("tqn", stage))
        wy, wt = C("wy", stage), C("wt", stage)
        si_off = (stage % 2) * 7 * T

        # trig (wrap inlined to use per-stage scratch)
        ts(dap(wy, [(1, 7)]), dap(qs, [(1, 7)]), 1.0 / TWO_PI, None)
        V.tensor_copy(SI[:, si_off:si_off + 7 * T], dap(wy, [(1, 7)]))
        V.tensor_copy(dap(wt, [(1, 7)]), SI[:, si_off:si_off + 7 * T])
        V.scalar_tensor_tensor(dap(sarg, [(1, 7)]), dap(wt, [(1, 7)]),
                               -TWO_PI, dap(qs, [(1, 7)]), AO.mult, AO.add)
        SC.activation(dap(sn, [(1, 7)]), dap(sarg, [(1, 7)]), ACT.Sin)
        SC.activation(dap(wt, [(1, 7)]), dap(sarg, [(1, 7)]), ACT.Abs)
        SC.activation(dap(cs, [(1, 7)]), dap(wt, [(1, 7)]), ACT.Sin,
                      bias=halfpi_ap, scale=-1.0)
        ts(dap(oc, [(1, 7)]), dap(cs, [(1, 7)]), -1.0, 1.0, AO.mult, AO.add)

        # R build into Mf[...,3:6]
        tt(dap(tb1, [(9, 7), (1, 9)]), dap(sn, [(1, 7), (0, 9)]),
           cap(co["K1n"], [(9, 7), (1, 9)]))
        tt(dap(Mf + 3, [(18, 7), (6, 3), (1, 3)]),
           cap(co["K0"], [(9, 7), (3, 3), (1, 3)]),
           dap(tb1, [(9, 7), (3, 3), (1, 3)]), AO.add)
        tt(dap(tb1, [(9, 7), (1, 9)]), dap(oc, [(1, 7), (0, 9)]),
           cap(co["K2"], [(9, 7), (1, 9)]))
        tt(dap(Mf + 3, [(18, 7), (6, 3), (1, 3)]),
           dap(Mf + 3, [(18, 7), (6, 3), (1, 3)]),
           dap(tb1, [(9, 7), (3, 3), (1, 3)]), AO.add)

        # pT build
        tt(dap(tb1, [(3, 7), (1, 3)]), dap(qs, [(1, 7), (0, 3)]),
           cap(co["e1"], [(3, 7), (1, 3)]))
        tt(dap(pT, [(1, 21)]), cap(co["e0"], [(1, 21)]),
           dap(tb1, [(1, 21)]), AO.add)
        for srcch, cname in ((sn, "e2"), (oc, "e3")):
            tt(dap(tb1, [(3, 7), (1, 3)]), dap(srcch, [(1, 7), (0, 3)]),
               cap(co[cname], [(3, 7), (1, 3)]))
            tt(dap(pT, [(1, 21)]), dap(pT, [(1, 21)]),
               dap(tb1, [(1, 21)]), AO.add)

        # p rolls
        cp(dap(pr1, [(3, 7), (1, 2)]), dap(pT + 1, [(3, 7), (1, 2)]))
        cp(dap(pr1 + 2, [(3, 7)]), dap(pT, [(3, 7)]))
        cp(dap(pr2, [(3, 7)]), dap(pT + 2, [(3, 7)]))
        cp(dap(pr2 + 1, [(3, 7), (1, 2)]), dap(pT, [(3, 7), (1, 2)]))

        # Q = hat(p) R into Mf[...,0:3]
        tt(dap(tqp, [(9, 7), (1, 3)]), dap(pr1, [(3, 7), (0, 3)]),
           dap(Mf + 2 * 6 + 3, [(18, 7), (1, 3)]))
        tt(dap(tqp + 3, [(9, 7), (1, 3)]), dap(pr1 + 1, [(3, 7), (0, 3)]),
           dap(Mf + 3, [(18, 7), (1, 3)]))
        tt(dap(tqp + 6, [(9, 7), (1, 3)]), dap(pr1 + 2, [(3, 7), (0, 3)]),
           dap(Mf + 6 + 3, [(18, 7), (1, 3)]))
        tt(dap(tqn, [(9, 7), (1, 3)]), dap(pr2, [(3, 7), (0, 3)]),
           dap(Mf + 1 * 6 + 3, [(18, 7), (1, 3)]))
        tt(dap(tqn + 3, [(9, 7), (1, 3)]), dap(pr2 + 1, [(3, 7), (0, 3)]),
           dap(Mf + 2 * 6 + 3, [(18, 7), (1, 3)]))
        tt(dap(tqn + 6, [(9, 7), (1, 3)]), dap(pr2 + 2, [(3, 7), (0, 3)]),
           dap(Mf + 3, [(18, 7), (1, 3)]))
        tt(dap(Mf, [(18, 7), (6, 3), (1, 3)]),
           dap(tqp, [(9, 7), (3, 3), (1, 3)]),
           dap(tqn, [(9, 7), (3, 3), (1, 3)]), AO.subtract)

    def emit_dyn(stage):
        dqs = dqs_blocks[stage]
        Mf = C("Mf", stage)
        pTb = C("pT", stage)
        # ---------- joint chain ----------
        # append all U_{i,i} = A_i slots up-front (constants, off-chain)
        cp(bass.AP(S.tensor, S.offset + (o["rows"] + 12 + 12) * T,
                   [list(S.ap[0]), [60 * T, 7], [1, 6 * T]]),
           cap(co["A"], [(6, 7), (1, 6)]), eng=V)
        for i in range(DOF):
            m = 2 + i
            pv = o["rows"] if i == 0 else o["rows"] + 12 + (i - 1) * 54
            ri = o["rows"] + 12 + i * 54
            mf = Mf + 18 * i
            # lower row: out comps 3..5 = [Q|R] . u6
            tt(dap(o["tpL"], [(18, m), (6, 3), (1, 6)]),
               dap(mf, [(0, m), (6, 3), (1, 6)]),
               dap(pv, [(6, m), (0, 3), (1, 6)]))
            red(dap(ri + 3, [(6, m), (1, 3)]),
                dap(o["tpL"], [(18, m), (6, 3), (1, 6)], bpos=2))
            # upper row: out comps 0..2 = R . uw
            tt(dap(o["tpU"], [(9, m), (3, 3), (1, 3)]),
               dap(mf + 3, [(0, m), (6, 3), (1, 3)]),
               dap(pv, [(6, m), (0, 3), (1, 3)]))
            red(dap(ri, [(6, m), (1, 3)]),
                dap(o["tpU"], [(9, m), (3, 3), (1, 3)], bpos=2))
            # ad(A_i) W_i (= ad(A_i) V_i since ad(A)A = 0), W = pre-add V
            tt(dap(o["tadv"], [(6, 6), (1, 6)]),
               cap(co["nadA"] + 36 * i, [(6, 6), (1, 6)]),
               dap(ri, [(0, 6), (1, 6)]))
            red(dap(o["adv12"] + 12 * i + 6, [(1, 6)]),
                dap(o["tadv"], [(6, 6), (1, 6)], bpos=1))
            # [V | Vd] += [A_i | adv] * dq_i  in one fused pair
            tt(dap(o["tv6"], [(1, 12)]), dap(o["adv12"] + 12 * i, [(1, 12)]),
               dap(dqs + i, [(0, 12)]))
            tt(dap(ri, [(1, 12)]), dap(ri, [(1, 12)]),
               dap(o["tv6"], [(1, 12)]), AO.add)

        rows0 = o["rows"] + 12

        # ---------- Ut = g * U (triangle) ----------
        for k in range(DOF):
            tt(dap(o["Ut"] + 42 * k, [(6, k + 1), (1, 6)]),
               cap(co["g"] + 6 * k, [(0, k + 1), (1, 6)]),
               dap(rows0 + 54 * k + 12, [(6, k + 1), (1, 6)]))

        # ---------- V rolls (w, v parts of Vs) ----------
        for dst, base, shift in ((o["wr1"], 0, 1), (o["wr2"], 0, 2),
                                 (o["vr1"], 3, 1), (o["vr2"], 3, 2)):
            n1 = 3 - shift
            cp(dap(dst, [(3, 7), (1, n1)]),
               dap(rows0 + base + shift, [(54, 7), (1, n1)]))
            cp(dap(dst + n1, [(3, 7), (1, shift)]),
               dap(rows0 + base, [(54, 7), (1, shift)]))

        # gwv1 = [gRr1|gm3] * [wr1|vr1] ; gwv2 = [gRr2|gm3] * [wr2|vr2]
        tt(dap(o["gwv1"], [(1, 42)]), cap(co["gA1"], [(1, 42)]),
           dap(o["wr1"], [(1, 42)]))
        tt(dap(o["gwv2"], [(1, 42)]), cap(co["gA2"], [(1, 42)]),
           dap(o["wr2"], [(1, 42)]))

        # ---------- Y = G Vd + (wxyw + vxyv ; wxyv) ----------
        # 4-combo cross products: [w,w,v,v] x [yw,yv,yw,yv] (one waste slot)
        tt(dap(o["Y"], [(6, 7), (1, 6)]), cap(co["g"], [(6, 7), (1, 6)]),
           dap(rows0 + 6, [(54, 7), (1, 6)]))
        tt(dap(o["tcp"], [(42, 2), (21, 2), (1, 21)]),
           dap(o["wr1"], [(21, 2), (0, 2), (1, 21)]),
           dap(o["gwv2"], [(0, 2), (21, 2), (1, 21)]))
        tt(dap(o["tcn"], [(42, 2), (21, 2), (1, 21)]),
           dap(o["wr2"], [(21, 2), (0, 2), (1, 21)]),
           dap(o["gwv1"], [(0, 2), (21, 2), (1, 21)]))
        tt(dap(o["tcs"], [(1, 84)]), dap(o["tcp"], [(1, 84)]),
           dap(o["tcn"], [(1, 84)]), AO.subtract)
        # Y_w += t[0] (wxyw) + t[3] (vxyv); Y_v += t[1] (wxyv)
        tt(dap(o["Y"], [(6, 7), (1, 3)]), dap(o["Y"], [(6, 7), (1, 3)]),
           dap(o["tcs"], [(3, 7), (1, 3)]), AO.add)
        tt(dap(o["Y"], [(6, 7), (1, 3)]), dap(o["Y"], [(6, 7), (1, 3)]),
           dap(o["tcs"] + 63, [(3, 7), (1, 3)]), AO.add)
        tt(dap(o["Y"] + 3, [(6, 7), (1, 3)]), dap(o["Y"] + 3, [(6, 7), (1, 3)]),
           dap(o["tcs"] + 21, [(3, 7), (1, 3)]), AO.add)

        # ---------- bias_i = sum_{k>=i} U_{k,i} . Y_k ----------
        for i in range(DOF):
            nk = DOF - i
            tt(dap(o["tbias"], [(6, nk), (1, 6)]),
               dap(rows0 + 54 * i + 12 + 6 * i, [(54, nk), (1, 6)]),
               dap(o["Y"] + 6 * i, [(6, nk), (1, 6)]))
            red(dap(o["bias"] + i, []),
                dap(o["tbias"], [(6, nk), (1, 6)], bpos=0), axis="X")
        # rhs (= qacc block) = tau - bias
        tt(dap(o["qacc"] + 7 * stage, [(1, 7)]), dap(o["tau"], [(1, 7)]),
           dap(o["bias"], [(1, 7)]), AO.subtract)

        # ---------- Gram mass matrix (lower triangle rows) ----------
        for j in range(DOF):
            nk = DOF - j
            ni = j + 1
            tt(dap(o["tgram"], [(6 * nk, ni), (6, nk), (1, 6)]),
               dap(rows0 + 54 * j + 12, [(6, ni), (54, nk), (1, 6)]),
               dap(o["Ut"] + 42 * j + 6 * j, [(0, ni), (42, nk), (1, 6)]))
            red(dap(o["Mg"] + 7 * j, [(1, ni)]),
                dap(o["tgram"], [(6 * nk, ni), (1, 6 * nk)], bpos=1))

        # ---------- LDL^T ----------
        rhs = o["qacc"] + 7 * stage
        for j in range(DOF):
            V.reciprocal(dap(o["rinv"] + j, []), dap(o["Mg"] + 8 * j, []))
            nr = DOF - 1 - j
            if nr == 0:
                break
            tt(dap(o["Lg"] + 7 * (j + 1) + j, [(7, nr)]),
               dap(o["Mg"] + 7 * (j + 1) + j, [(7, nr)]),
               dap(o["rinv"] + j, [(0, nr)]))
            tt(dap(o["tso"], [(nr, nr), (1, nr)]),
               dap(o["Lg"] + 7 * (j + 1) + j, [(7, nr), (0, nr)]),
               dap(o["Mg"] + 7 * (j + 1) + j, [(0, nr), (7, nr)]))
            tt(dap(o["Mg"] + 8 * (j + 1), [(7, nr), (1, nr)]),
               dap(o["Mg"] + 8 * (j + 1), [(7, nr), (1, nr)]),
               dap(o["tso"], [(nr, nr), (1, nr)]), AO.subtract)
        for j in range(DOF - 1):
            nr = DOF - 1 - j
            tt(dap(o["tss"], [(1, nr)]),
               dap(o["Lg"] + 7 * (j + 1) + j, [(7, nr)]),
               dap(rhs + j, [(0, nr)]))
            tt(dap(rhs + j + 1, [(1, nr)]), dap(rhs + j + 1, [(1, nr)]),
               dap(o["tss"], [(1, nr)]), AO.subtract)
        tt(dap(rhs, [(1, 7)]), dap(rhs, [(1, 7)]), dap(o["rinv"], [(1, 7)]))
        for j in range(DOF - 1, 0, -1):
            tt(dap(o["tss"], [(1, j)]), dap(o["Lg"] + 7 * j, [(1, j)]),
               dap(rhs + j, [(0, j)]))
            tt(dap(rhs, [(1, j)]), dap(rhs, [(1, j)]),
               dap(o["tss"], [(1, j)]), AO.subtract)


    # schedule: builds as early as deps allow, overlapping prior dynamics
    emit_build(0)
    prep_qs(1)
    emit_build(1)
    emit_dyn(0)
    prep_dqs(1)
    prep_qs(2)
    emit_build(2)
    emit_dyn(1)
    prep_dqs(2)
    prep_qs(3)
    emit_build(3)
    emit_dyn(2)
    prep_dqs(3)
    emit_dyn(3)

    # ---------- RK4 combine ----------
    # qn = q0 + H/6 * sum w_s*dqs_s ; dqn = dq0 + H/6 * sum w_s*qacc_s
    tt(dap(o["tw"], [(7, 4), (1, 7)]), cap(co["w4"], [(1, 4), (0, 7)]),
       dap(o["dq0"], [(7, 4), (1, 7)]))
    red(dap(o["tk7"], [(1, 7)]), dap(o["tw"], [(1, 7), (7, 4)], bpos=1))
    V.scalar_tensor_tensor(dap(o["qn"], [(1, 7)]), dap(o["tk7"], [(1, 7)]),
        H / 6.0, dap(o["q0"], [(1, 7)]), AO.mult, AO.add)
    tt(dap(o["tw"], [(7, 4), (1, 7)]), cap(co["w4"], [(1, 4), (0, 7)]),
       dap(o["qacc"], [(7, 4), (1, 7)]))
    red(dap(o["tk7"], [(1, 7)]), dap(o["tw"], [(1, 7), (7, 4)], bpos=1))
    V.scalar_tensor_tensor(dap(o["dqn"], [(1, 7)]), dap(o["tk7"], [(1, 7)]),
        H / 6.0, dap(o["dq0"], [(1, 7)]), AO.mult, AO.add)

    # wrap q -> outch[0:7]
    wrap_to(o["outch"], o["qn"])
    # clip dq -> outch[7:14]
    ts(dap(o["outch"] + 7, [(1, 7)]), dap(o["dqn"], [(1, 7)]),
       -MAX_VELOCITY, MAX_VELOCITY, AO.max, AO.min)

    # ---------- FK ----------
    qw = o["outch"]
    sincos(o["s2"], o["c2"], qw)
    ts(dap(o["oc2"], [(1, 7)]), dap(o["c2"], [(1, 7)]), -1.0, 1.0,
       AO.mult, AO.add)

    tt(dap(o["tb1"], [(9, 7), (1, 9)]), dap(o["s2"], [(1, 7), (0, 9)]),
       cap(co["F1"], [(9, 7), (1, 9)]))
    tt(dap(o["RB"], [(1, 63)]), cap(co["F0"], [(1, 63)]),
       dap(o["tb1"], [(1, 63)]), AO.add)
    tt(dap(o["tb1"], [(9, 7), (1, 9)]), dap(o["oc2"], [(1, 7), (0, 9)]),
       cap(co["F2"], [(9, 7), (1, 9)]))
    tt(dap(o["RB"], [(1, 63)]), dap(o["RB"], [(1, 63)]),
       dap(o["tb1"], [(1, 63)]), AO.add)

    tt(dap(o["tb1"], [(3, 7), (1, 3)]), dap(qw, [(1, 7), (0, 3)]),
       cap(co["f1"], [(3, 7), (1, 3)]))
    tt(dap(o["pB"], [(1, 21)]), cap(co["f0"], [(1, 21)]),
       dap(o["tb1"], [(1, 21)]), AO.add)
    for src, cname in ((o["s2"], "f2"), (o["oc2"], "f3")):
        tt(dap(o["tb1"], [(3, 7), (1, 3)]), dap(src, [(1, 7), (0, 3)]),
           cap(co[cname], [(3, 7), (1, 3)]))
        tt(dap(o["pB"], [(1, 21)]), dap(o["pB"], [(1, 21)]),
           dap(o["tb1"], [(1, 21)]), AO.add)

    cp(dap(o["rcur"], [(1, 6)]), cap(co["r01i"], [(1, 6)]))
    V.memset(dap(o["outch"] + 14, [(1, 2)]), 0.0)
    cur, nxt = o["rcur"], o["rnxt"]
    for i in range(DOF):
        # p01 += rows . pB_i
        tt(dap(o["tfd"], [(3, 2), (1, 3)]), dap(cur, [(3, 2), (1, 3)]),
           dap(o["pB"] + 3 * i, [(0, 2), (1, 3)]))
        red(dap(o["tpd"], [(1, 2)]), dap(o["tfd"], [(3, 2), (1, 3)], bpos=1))
        tt(dap(o["outch"] + 14, [(1, 2)]), dap(o["outch"] + 14, [(1, 2)]),
           dap(o["tpd"], [(1, 2)]), AO.add)
        # rows = rows @ RB_i  (per row; P[c][c'] = r[c]*RB[c][c'])
        for row in range(2):
            tt(dap(o["tfr"] + 9 * row, [(3, 3), (1, 3)]),
               dap(cur + 3 * row, [(1, 3), (0, 3)]),
               dap(o["RB"] + 9 * i, [(3, 3), (1, 3)]))
            red(dap(nxt + 3 * row, [(1, 3)]),
                dap(o["tfr"] + 9 * row, [(1, 3), (3, 3)], bpos=1))
        cur, nxt = nxt, cur
    tt(dap(o["tfd"], [(3, 2), (1, 3)]), dap(cur, [(3, 2), (1, 3)]),
       cap(co["pe"], [(0, 2), (1, 3)]))
    red(dap(o["tpd"], [(1, 2)]), dap(o["tfd"], [(3, 2), (1, 3)], bpos=1))
    tt(dap(o["outch"] + 14, [(1, 2)]), dap(o["outch"] + 14, [(1, 2)]),
       dap(o["tpd"], [(1, 2)]), AO.add)

    # ---------- interleave + store ----------
    cp(bass.AP(S.tensor, S.offset + o["stag_out"] * T,
               [list(S.ap[0])] + [[1, 16], [16, T]]),
       dap(o["outch"], [(1, 16)], bpos=1))
    nc.sync.dma_start(out_d[:].rearrange("(p t) c -> p (t c)", p=P),
                      S[:, o["stag_out"] * T:(o["stag_out"] + 16) * T])


def _split_waits(nc, maxw=1):
    """This walrus build accepts at most one embedded sync-wait per
    instruction; hoist extras into standalone NoOps on the same engine."""
    from concourse import mybir

    for fn in nc.m.functions:
        for blk in fn.blocks:
            new = []
            for inst in blk.instructions:
                si = getattr(inst, "sync_info", None)
                if si is not None and si.on_wait and len(si.on_wait) > maxw:
                    waits = list(si.on_wait)
                    for k, w in enumerate(waits[maxw:]):
                        new.append(mybir.InstNoOp(
                            name=f"{inst.name}-w{k}",
                            sync_info=mybir.SyncInfo(on_wait=[w], on_update=[]),
                            engine=inst.engine,
                        ))
                    si.on_wait = waits[:maxw]
                new.append(inst)
            blk.instructions = new


# ============================ host entry ============================
_BUILT = None


def _build():
    global _BUILT
    if _BUILT is not None:
        return _BUILT
    import concourse.bass as bass
    import concourse.tile as tile
    from concourse import mybir

    f32 = mybir.dt.float32
    nc = bass.Bass()
    state_d = nc.declare_dram_parameter("state", [PB, 14], f32, isOutput=False)
    action_d = nc.declare_dram_parameter("action", [PB, 7], f32,
                                         isOutput=False)
    consts_d = nc.declare_dram_parameter("consts", [P, NCOL], f32,
                                         isOutput=False)
    out_d = nc.declare_dram_parameter("out", [PB, 16], f32, isOutput=True)
    with tile.TileContext(nc) as tc:
        with tc.tile_pool(name="main", bufs=1) as pool:
            emit(nc, tc, pool, state_d, action_d, consts_d, out_d)
    _split_waits(nc)
    _BUILT = nc
    return nc


def run_cores(state, action, consts, trace=False):
    from concourse.bass_utils import run_bass_kernel_spmd

    nc = _build()
    state = np.ascontiguousarray(np.asarray(state, np.float32))
    action = np.ascontiguousarray(np.asarray(action, np.float32))
    in_maps = []
    for c in range(NCORES):
        sl = slice(c * PB, (c + 1) * PB)
        in_maps.append({"state": state[sl], "action": action[sl],
                        "consts": consts})
    res = run_bass_kernel_spmd(nc, in_maps, core_ids=list(range(NCORES)),
                               trace=trace)
    out = np.concatenate([res.results[i]["out"] for i in range(NCORES)], 0)
    return out, res


def kernel(state, action, _M, _A, _G, gravity):
    consts = make_consts(_M, _A, _G, gravity)
    out, _ = run_cores(state, action, consts)
    return out[:, :14].copy(), out[:, 14:16].copy()


# revision 22
# speedup vs baseline: 1.0121x; 1.0121x over previous
"""Trainium2 Bass kernel for nn_ArmModel (7-DOF arm RK4 dynamics step + FK).

Self-contained: hardcodes shapes/sharding. 8-core pure data parallelism over
the batch (16384 -> 2048/core). Per core, batch is laid out as 128 partitions
x 16 free columns; every scalar quantity ("channel") is a (128,16) block at a
free-dim offset of one big SBUF tile. All math is emitted as vector/scalar
engine ops with strided/broadcast access patterns.

Algorithm (mathematically identical to the reference, restructured):
  T_i(q)   = exp(-q_i [A_i]) @ Minv_i gives Ad = [[R,0],[Q,R]], Q = hat(p) R
  R        = K0 - s*K1 + (1-c)*K2           (K* host-precomputed per joint)
  p        = e0 + q*e1 + s*e2 + (1-c)*e3
  forward:  bundle [V, Vd, U_j] propagated by Ad per joint;
            Vd += -ad(A_i) V_i * dq_i ; V += A_i dq_i
  bias_i   = sum_{k>=i} U_{k,i} . Y_k,  Y_k = G Vd_k - ad^T(V_k) G V_k
  M_{ij}   = sum_{k>=max(i,j)} U_{k,i} . (G_k U_{k,j})   (Gram form)
  solve    = LDL^T (no pivoting; M SPD)
  RK4, wrap q, clip dq, FK rows-accumulation for end-effector (x,y).
"""
import numpy as np

DOF = 7
BATCH = 16384
NCORES = 8
PB = BATCH // NCORES      # 2048 per core
P = 128
T = 16                    # batch cols per channel; PB = P*T
H = 0.1
ACTION_RANGE = 50.0
MAX_VELOCITY = 20.0

F32 = None  # set lazily (mybir.dt.float32)


# ============================ constant layout ============================
class ConstLayout:
    def __init__(self):
        self.off = {}
        self.n = 0

    def alloc(self, name, n):
        self.off[name] = self.n
        self.n += n
        return self.off[name]


CL = ConstLayout()
CL.alloc("A", 42)          # [j*6+c]
CL.alloc("nadA", 252)      # [j*36 + r*6 + c]   (-ad(A_j))
CL.alloc("g", 42)          # [k*6+c]
CL.alloc("gRr1", 21)       # [k*3+c] = gI[p1(c)]
CL.alloc("gRr2", 21)
CL.alloc("gm3", 21)        # [k*3+c] = mass scalar replicated
CL.alloc("K0", 63)         # [j*9 + r*3 + c]
CL.alloc("K1n", 63)        # -K1
CL.alloc("K2", 63)
CL.alloc("e0", 21)
CL.alloc("e1", 21)
CL.alloc("e2", 21)
CL.alloc("e3", 21)
CL.alloc("F0", 63)
CL.alloc("F1", 63)
CL.alloc("F2", 63)
CL.alloc("f0", 21)
CL.alloc("f1", 21)
CL.alloc("f2", 21)
CL.alloc("f3", 21)
CL.alloc("pe", 3)
CL.alloc("vd0", 6)         # (0,0,0,-gravity)
CL.alloc("r01i", 6)        # rows init (1,0,0),(0,1,0)
CL.alloc("w4", 4)          # RK weights 1,2,2,1
CL.alloc("halfpi", 1)
CL.alloc("gA1", 42)        # [gRr1 | gm3]
CL.alloc("gA2", 42)        # [gRr2 | gm3]
NCOL = CL.n


def _hat(w):
    return np.array([[0, -w[2], w[1]], [w[2], 0, -w[0]], [-w[1], w[0], 0]],
                    np.float64)


def make_consts(_M, _A, _G, gravity):
    """Host-side: build the (128, NCOL) f32 constant tile."""
    _M = np.asarray(_M, np.float64)
    _A = np.asarray(_A, np.float64)
    _G = np.asarray(_G, np.float64)
    gravity = np.asarray(gravity, np.float64)
    buf = np.zeros((NCOL,), np.float64)
    o = CL.off
    p1 = [1, 2, 0]
    p2 = [2, 0, 1]

    Ms = []
    for k in range(DOF + 1):
        a1, a2, p = _M[k, :3, 0], _M[k, :3, 1], _M[k, :3, 3]
        b1 = a1 / np.linalg.norm(a1)
        a2o = a2 - (a2 @ b1) * b1
        b2 = a2o / np.linalg.norm(a2o)
        b3 = np.cross(b1, b2)
        R = np.stack([b1, b2, b3], -1)
        Tm = np.eye(4)
        Tm[:3, :3] = R
        Tm[:3, 3] = p
        Ms.append(Tm)

    gabs = np.abs(_G)
    for i in range(DOF):
        w = _A[i, :3] / np.linalg.norm(_A[i, :3])
        v = _A[i, 3:]
        A6 = np.concatenate([w, v])
        W = _hat(w)
        W2 = W @ W
        R_, p_ = Ms[i][:3, :3], Ms[i][:3, 3]
        Rm, pm = R_.T, -R_.T @ p_

        buf[o["A"] + 6 * i: o["A"] + 6 * i + 6] = A6
        # ad(A) = [[hat(w),0],[hat(v),hat(w)]]; store negated
        adA = np.zeros((6, 6))
        adA[:3, :3] = W
        adA[3:, 3:] = W
        adA[3:, :3] = _hat(v)
        buf[o["nadA"] + 36 * i: o["nadA"] + 36 * (i + 1)] = (-adA).ravel()

        gd = np.concatenate([gabs[i, :3], np.repeat(gabs[i, 3], 3)])
        buf[o["g"] + 6 * i: o["g"] + 6 * i + 6] = gd
        buf[o["gRr1"] + 3 * i: o["gRr1"] + 3 * i + 3] = gd[:3][p1]
        buf[o["gRr2"] + 3 * i: o["gRr2"] + 3 * i + 3] = gd[:3][p2]
        buf[o["gm3"] + 3 * i: o["gm3"] + 3 * i + 3] = gd[3]

        buf[o["K0"] + 9 * i: o["K0"] + 9 * (i + 1)] = Rm.ravel()
        buf[o["K1n"] + 9 * i: o["K1n"] + 9 * (i + 1)] = (-(W @ Rm)).ravel()
        buf[o["K2"] + 9 * i: o["K2"] + 9 * (i + 1)] = (W2 @ Rm).ravel()
        buf[o["e0"] + 3 * i: o["e0"] + 3 * i + 3] = pm
        buf[o["e1"] + 3 * i: o["e1"] + 3 * i + 3] = -(v + W2 @ v)
        buf[o["e2"] + 3 * i: o["e2"] + 3 * i + 3] = W2 @ v - W @ pm
        buf[o["e3"] + 3 * i: o["e3"] + 3 * i + 3] = W2 @ pm + W @ v

        RM, pM = Ms[i][:3, :3], Ms[i][:3, 3]
        buf[o["F0"] + 9 * i: o["F0"] + 9 * (i + 1)] = RM.ravel()
        buf[o["F1"] + 9 * i: o["F1"] + 9 * (i + 1)] = (RM @ W).ravel()
        buf[o["F2"] + 9 * i: o["F2"] + 9 * (i + 1)] = (RM @ W2).ravel()
        buf[o["f0"] + 3 * i: o["f0"] + 3 * i + 3] = pM
        buf[o["f1"] + 3 * i: o["f1"] + 3 * i + 3] = RM @ (v + W2 @ v)
        buf[o["f2"] + 3 * i: o["f2"] + 3 * i + 3] = -RM @ (W2 @ v)
        buf[o["f3"] + 3 * i: o["f3"] + 3 * i + 3] = RM @ (W @ v)

    buf[o["pe"]: o["pe"] + 3] = Ms[DOF][:3, 3]
    buf[o["vd0"] + 3: o["vd0"] + 6] = -gravity
    buf[o["r01i"]: o["r01i"] + 6] = [1, 0, 0, 0, 1, 0]
    buf[o["w4"]: o["w4"] + 4] = [1, 2, 2, 1]
    buf[o["halfpi"]] = np.pi / 2
    buf[o["gA1"]: o["gA1"] + 21] = buf[o["gRr1"]: o["gRr1"] + 21]
    buf[o["gA1"] + 21: o["gA1"] + 42] = buf[o["gm3"]: o["gm3"] + 21]
    buf[o["gA2"]: o["gA2"] + 21] = buf[o["gRr2"]: o["gRr2"] + 21]
    buf[o["gA2"] + 21: o["gA2"] + 42] = buf[o["gm3"]: o["gm3"] + 21]
    return np.tile(buf.astype(np.float32)[None, :], (P, 1))


# ============================ channel layout ============================
class ChLayout:
    def __init__(self):
        self.off = {}
        self.n = 0

    def alloc(self, name, n):
        self.off[name] = self.n
        self.n += n
        return self.off[name]


CH = ChLayout()
CH.alloc("stag_in", 14)     # state staging [t][c] interleaved
CH.alloc("stag_act", 7)
CH.alloc("q0", 7)
CH.alloc("dq0", 7)          # == dqs block s=0 (must follow q0)
CH.alloc("dqs1", 7)
CH.alloc("dqs2", 7)
CH.alloc("dqs3", 7)
CH.alloc("qacc", 28)        # 4 stages x 7 (used as rhs/z during solve)
CH.alloc("tau", 7)
CH.alloc("qs", 7)
CH.alloc("sarg", 7)
CH.alloc("carg", 7)
CH.alloc("sn", 7)
CH.alloc("cs", 7)
CH.alloc("oc", 7)
CH.alloc("Mf", 126)         # [j*18 + r*6 + c6]; c6 0..2 = Q, 3..5 = R
CH.alloc("pT", 21)
CH.alloc("pr1", 21)
CH.alloc("pr2", 21)
CH.alloc("tb1", 63)
CH.alloc("tqp", 63)
CH.alloc("tqn", 63)
CH.alloc("rows", 12 + 7 * 54)   # init [V,Vd] then per joint [V,Vd,U_0..U_6]
CH.alloc("Ut", 294)         # [k*42 + j*6 + c]
CH.alloc("wr1", 21)
CH.alloc("vr1", 21)
CH.alloc("wr2", 21)
CH.alloc("vr2", 21)
CH.alloc("gwv1", 42)
CH.alloc("gwv2", 42)
CH.alloc("tcp", 84)
CH.alloc("tcn", 84)
CH.alloc("tcs", 84)
CH.alloc("adv12", 84)
CH.alloc("Y", 42)
CH.alloc("pos1", 21)
CH.alloc("neg1", 21)
CH.alloc("pos2", 21)
CH.alloc("neg2", 21)
CH.alloc("pos3", 21)
CH.alloc("neg3", 21)
CH.alloc("tc1", 21)
CH.alloc("bias", 7)
CH.alloc("Mg", 49)          # mass matrix grid [i*7+j] (lower valid)
CH.alloc("Lg", 49)
CH.alloc("rinv", 7)
CH.alloc("tpL", 144)
CH.alloc("tpU", 72)
CH.alloc("tadv", 36)
CH.alloc("adv", 6)
CH.alloc("advb", 6)
CH.alloc("tv6", 12)
CH.alloc("tgram", 96)
CH.alloc("tg2", 16)
CH.alloc("wy", 7)
CH.alloc("wt", 7)
CH.alloc("wm", 7)
CH.alloc("tbias", 42)
CH.alloc("tso", 36)
CH.alloc("tss", 7)
CH.alloc("tw", 28)
CH.alloc("tk7", 7)
CH.alloc("qn", 7)
CH.alloc("dqn", 7)
CH.alloc("outch", 16)       # q(7) dq(7) px py
CH.alloc("RB", 63)
CH.alloc("pB", 21)
CH.alloc("s2", 7)
CH.alloc("c2", 7)
CH.alloc("oc2", 7)
CH.alloc("ca2", 7)
CH.alloc("oc2t", 7)
CH.alloc("rcur", 6)
CH.alloc("rnxt", 6)
CH.alloc("tfr", 18)
CH.alloc("tfd", 6)
CH.alloc("tpd", 2)
CH.alloc("stag_out", 16)
NCH = CH.n


# ============================ emit ============================
def emit(nc, tc, pool, state_d, action_d, consts_d, out_d):
    import concourse.bass as bass
    from concourse import mybir

    f32 = mybir.dt.float32
    AO = mybir.AluOpType
    ACT = mybir.ActivationFunctionType
    S = pool.tile([P, NCH * T], f32)
    CT = pool.tile([P, NCOL], f32)
    SI = pool.tile([P, 7 * T], mybir.dt.int32)
    o = CH.off
    co = CL.off

    def _merge(free):
        out = []
        for st, n in free:
            if out and out[-1][0] == st * n:
                out[-1] = [st, n * out[-1][1]]
            else:
                out.append([st, n])
        assert len(out) <= 3, f"AP has {len(out)} free dims after merge: {out}"
        return out

    def dap(ch, dims=(), bpos=None):
        """Data AP. dims: list of (step_in_channels, count). Batch dim [1,T]
        appended last unless bpos gives its index within dims. Adjacent
        contiguous dims are merged (HW allows max 3 free dims)."""
        free = [[st * T, n] for st, n in dims]
        if bpos is None:
            free = free + [[1, T]]
        else:
            free = free[:bpos] + [[1, T]] + free[bpos:]
        return bass.AP(S.tensor, S.offset + ch * T,
                       [list(S.ap[0])] + _merge(free))

    def cap(col, dims=(), bpos=None):
        free = [[st, n] for st, n in dims]
        if bpos is None:
            free = free + [[0, T]]
        else:
            free = free[:bpos] + [[0, T]] + free[bpos:]
        return bass.AP(CT.tensor, CT.offset + col,
                       [list(CT.ap[0])] + _merge(free))

    V = nc.vector
    SC = nc.scalar
    G = nc.gpsimd

    def tt(out, a, b, op=AO.mult, eng=V):
        eng.tensor_tensor(out, a, b, op)

    def ts(out, a, s1, s2=None, op0=AO.mult, op1=AO.add, eng=V):
        if s2 is None:
            eng.tensor_scalar(out, a, s1, None, op0)
        else:
            eng.tensor_scalar(out, a, s1, s2, op0, op1)

    def red(out, a, axis="X", op=AO.add):
        V.tensor_reduce(out, a, getattr(mybir.AxisListType, axis), op)

    def cp(out, a, eng=None):
        V.tensor_copy(out, a)

    PI = float(np.pi)
    TWO_PI = float(2 * np.pi)

    halfpi_ap = bass.AP(CT.tensor, CT.offset + co["halfpi"],
                        [list(CT.ap[0]), [0, 1]])

    def wrap_to(out_ch, x_ch, n=7):
        """out = x - 2pi*round(x/2pi), in [-pi, pi]. The HW f32->i32 cast
        rounds to nearest-even; floor((x+pi)/2pi) == round(x/2pi) away from
        ties, so this matches the reference mod semantics."""
        ts(dap(o["wy"], [(1, n)]), dap(x_ch, [(1, n)]), 1.0 / TWO_PI, None)
        V.tensor_copy(SI[:, :n * T], dap(o["wy"], [(1, n)]))
        V.tensor_copy(dap(o["wt"], [(1, n)]), SI[:, :n * T])
        V.scalar_tensor_tensor(dap(out_ch, [(1, n)]), dap(o["wt"], [(1, n)]),
                               -TWO_PI, dap(x_ch, [(1, n)]), AO.mult, AO.add)

    def sincos(s_ch, c_ch, arg_ch, n=7):
        """s = sin(arg), c = cos(arg) for arg in [-pi, pi]. cos via
        sin(pi/2 - |arg|) so the ACT input stays in range."""
        SC.activation(dap(s_ch, [(1, n)]), dap(arg_ch, [(1, n)]), ACT.Sin)
        SC.activation(dap(o["wm"], [(1, n)]), dap(arg_ch, [(1, n)]), ACT.Abs)
        SC.activation(dap(c_ch, [(1, n)]), dap(o["wm"], [(1, n)]), ACT.Sin,
                      bias=halfpi_ap, scale=-1.0)

    # ---------- load inputs ----------
    nc.sync.dma_start(S[:, o["stag_in"] * T:(o["stag_in"] + 14) * T],
                      state_d[:].rearrange("(p t) c -> p (t c)", p=P))
    nc.sync.dma_start(S[:, o["stag_act"] * T:(o["stag_act"] + 7) * T],
                      action_d[:].rearrange("(p t) c -> p (t c)", p=P))
    nc.sync.dma_start(CT[:], consts_d[:])

    # de-interleave: [t][c] -> channel-major q0(7),dq0(7)
    cp(dap(o["q0"], [(1, 14)], bpos=1),
       bass.AP(S.tensor, S.offset + o["stag_in"] * T,
               [list(S.ap[0])] + [[1, 14], [14, T]]))
    # tau = action * 50
    ts(dap(o["tau"], [(1, 7)], bpos=1),
       bass.AP(S.tensor, S.offset + o["stag_act"] * T,
               [list(S.ap[0])] + [[1, 7], [7, T]]),
       ACTION_RANGE, None)

    # init row: V=0, Vd=(0,0,0,-gravity)
    V.memset(dap(o["rows"], [(1, 6)]), 0.0)
    cp(dap(o["rows"] + 6, [(1, 6)]), cap(co["vd0"], [(1, 6)]))
    # memset M grid upper-garbage guard + U grid (avoid NaN reads in sim)
    V.memset(dap(o["Mg"], [(1, 49)]), 0.0)

    # adv12 blocks: [A_j (6) | adv_j (6)] per joint; A halves are static
    cp(bass.AP(S.tensor, S.offset + o["adv12"] * T,
               [list(S.ap[0]), [12 * T, 7], [1, 6 * T]]),
       cap(co["A"], [(6, 7), (1, 6)]), eng=V)

    dqs_blocks = [o["dq0"], o["dqs1"], o["dqs2"], o["dqs3"]]
    qs_ch = [o["q0"], o["qs"], o["qs"], o["qs"]]

    def C(name, stage):
        return o[name]

    def prep_qs(stage):
        # qs_s = q0 + a_h*dqs_{s-1}; depends on solve_{s-2} only
        a_h = [None, 0.5 * H, 0.5 * H, H][stage]
        V.scalar_tensor_tensor(dap(qs_ch[stage], [(1, 7)]),
            dap(dqs_blocks[stage - 1], [(1, 7)]), a_h,
            dap(o["q0"], [(1, 7)]), AO.mult, AO.add)

    def prep_dqs(stage):
        a_h = [None, 0.5 * H, 0.5 * H, H][stage]
        V.scalar_tensor_tensor(dap(dqs_blocks[stage], [(1, 7)]),
            dap(o["qacc"] + 7 * (stage - 1), [(1, 7)]), a_h,
            dap(o["dq0"], [(1, 7)]), AO.mult, AO.add)

    def emit_build(stage):
        qs = qs_ch[stage]
        sarg, sn, cs, oc = (C("sarg", stage), C("sn", stage), C("cs", stage),
                            C("oc", stage))
        tb1, Mf, pT = C("tb1", stage), C("Mf", stage), C("pT", stage)
        pr1, pr2, tqp, tqn = (C("pr1", stage), C("pr2", stage),
                              C("tqp", stage), C("tqn", stage))
        wy, wt = C("wy", stage), C("wt", stage)
        si_off = 0

        # trig (wrap inlined to use per-stage scratch)
        ts(dap(wy, [(1, 7)]), dap(qs, [(1, 7)]), 1.0 / TWO_PI, None)
        V.tensor_copy(SI[:, si_off:si_off + 7 * T], dap(wy, [(1, 7)]))
        V.tensor_copy(dap(wt, [(1, 7)]), SI[:, si_off:si_off + 7 * T])
        V.scalar_tensor_tensor(dap(sarg, [(1, 7)]), dap(wt, [(1, 7)]),
                               -TWO_PI, dap(qs, [(1, 7)]), AO.mult, AO.add)
        SC.activation(dap(sn, [(1, 7)]), dap(sarg, [(1, 7)]), ACT.Sin)
        SC.activation(dap(wt, [(1, 7)]), dap(sarg, [(1, 7)]), ACT.Abs)
        SC.activation(dap(cs, [(1, 7)]), dap(wt, [(1, 7)]), ACT.Sin,
                      bias=halfpi_ap, scale=-1.0)
        ts(dap(oc, [(1, 7)]), dap(cs, [(1, 7)]), -1.0, 1.0, AO.mult, AO.add)

        # R build into Mf[...,3:6]
        tt(dap(tb1, [(9, 7), (1, 9)]), dap(sn, [(1, 7), (0, 9)]),
           cap(co["K1n"], [(9, 7), (1, 9)]))
        tt(dap(Mf + 3, [(18, 7), (6, 3), (1, 3)]),
           cap(co["K0"], [(9, 7), (3, 3), (1, 3)]),
           dap(tb1, [(9, 7), (3, 3), (1, 3)]), AO.add)
        tt(dap(tb1, [(9, 7), (1, 9)]), dap(oc, [(1, 7), (0, 9)]),
           cap(co["K2"], [(9, 7), (1, 9)]))
        tt(dap(Mf + 3, [(18, 7), (6, 3), (1, 3)]),
           dap(Mf + 3, [(18, 7), (6, 3), (1, 3)]),
           dap(tb1, [(9, 7), (3, 3), (1, 3)]), AO.add)

        # pT build
        tt(dap(tb1, [(3, 7), (1, 3)]), dap(qs, [(1, 7), (0, 3)]),
           cap(co["e1"], [(3, 7), (1, 3)]))
        tt(dap(pT, [(1, 21)]), cap(co["e0"], [(1, 21)]),
           dap(tb1, [(1, 21)]), AO.add)
        for srcch, cname in ((sn, "e2"), (oc, "e3")):
            tt(dap(tb1, [(3, 7), (1, 3)]), dap(srcch, [(1, 7), (0, 3)]),
               cap(co[cname], [(3, 7), (1, 3)]))
            tt(dap(pT, [(1, 21)]), dap(pT, [(1, 21)]),
               dap(tb1, [(1, 21)]), AO.add)

        # p rolls
        cp(dap(pr1, [(3, 7), (1, 2)]), dap(pT + 1, [(3, 7), (1, 2)]))
        cp(dap(pr1 + 2, [(3, 7)]), dap(pT, [(3, 7)]))
        cp(dap(pr2, [(3, 7)]), dap(pT + 2, [(3, 7)]))
        cp(dap(pr2 + 1, [(3, 7), (1, 2)]), dap(pT, [(3, 7), (1, 2)]))

        # Q = hat(p) R into Mf[...,0:3]
        tt(dap(tqp, [(9, 7), (1, 3)]), dap(pr1, [(3, 7), (0, 3)]),
           dap(Mf + 2 * 6 + 3, [(18, 7), (1, 3)]))
        tt(dap(tqp + 3, [(9, 7), (1, 3)]), dap(pr1 + 1, [(3, 7), (0, 3)]),
           dap(Mf + 3, [(18, 7), (1, 3)]))
        tt(dap(tqp + 6, [(9, 7), (1, 3)]), dap(pr1 + 2, [(3, 7), (0, 3)]),
           dap(Mf + 6 + 3, [(18, 7), (1, 3)]))
        tt(dap(tqn, [(9, 7), (1, 3)]), dap(pr2, [(3, 7), (0, 3)]),
           dap(Mf + 1 * 6 + 3, [(18, 7), (1, 3)]))
        tt(dap(tqn + 3, [(9, 7), (1, 3)]), dap(pr2 + 1, [(3, 7), (0, 3)]),
           dap(Mf + 2 * 6 + 3, [(18, 7), (1, 3)]))
        tt(dap(tqn + 6, [(9, 7), (1, 3)]), dap(pr2 + 2, [(3, 7), (0, 3)]),
           dap(Mf + 3, [(18, 7), (1, 3)]))
        tt(dap(Mf, [(18, 7), (6, 3), (1, 3)]),
           dap(tqp, [(9, 7), (3, 3), (1, 3)]),
           dap(tqn, [(9, 7), (3, 3), (1, 3)]), AO.subtract)

    def emit_dyn(stage):
        dqs = dqs_blocks[stage]
        Mf = C("Mf", stage)
        pTb = C("pT", stage)
        # ---------- joint chain ----------
        # append all U_{i,i} = A_i slots up-front (constants, off-chain)
        cp(bass.AP(S.tensor, S.offset + (o["rows"] + 12 + 12) * T,
                   [list(S.ap[0]), [60 * T, 7], [1, 6 * T]]),
           cap(co["A"], [(6, 7), (1, 6)]), eng=V)
        for i in range(DOF):
            m = 2 + i
            pv = o["rows"] if i == 0 else o["rows"] + 12 + (i - 1) * 54
            ri = o["rows"] + 12 + i * 54
            mf = Mf + 18 * i
            # lower row: out comps 3..5 = [Q|R] . u6
            tt(dap(o["tpL"], [(18, m), (6, 3), (1, 6)]),
               dap(mf, [(0, m), (6, 3), (1, 6)]),
               dap(pv, [(6, m), (0, 3), (1, 6)]))
            red(dap(ri + 3, [(6, m), (1, 3)]),
                dap(o["tpL"], [(18, m), (6, 3), (1, 6)], bpos=2))
            # upper row: out comps 0..2 = R . uw
            tt(dap(o["tpU"], [(9, m), (3, 3), (1, 3)]),
               dap(mf + 3, [(0, m), (6, 3), (1, 3)]),
               dap(pv, [(6, m), (0, 3), (1, 3)]))
            red(dap(ri, [(6, m), (1, 3)]),
                dap(o["tpU"], [(9, m), (3, 3), (1, 3)], bpos=2))
            # ad(A_i) W_i (= ad(A_i) V_i since ad(A)A = 0), W = pre-add V
            tt(dap(o["tadv"], [(6, 6), (1, 6)]),
               cap(co["nadA"] + 36 * i, [(6, 6), (1, 6)]),
               dap(ri, [(0, 6), (1, 6)]))
            red(dap(o["adv12"] + 12 * i + 6, [(1, 6)]),
                dap(o["tadv"], [(6, 6), (1, 6)], bpos=1))
            # [V | Vd] += [A_i | adv] * dq_i  in one fused pair
            tt(dap(o["tv6"], [(1, 12)]), dap(o["adv12"] + 12 * i, [(1, 12)]),
               dap(dqs + i, [(0, 12)]))
            tt(dap(ri, [(1, 12)]), dap(ri, [(1, 12)]),
               dap(o["tv6"], [(1, 12)]), AO.add)

        rows0 = o["rows"] + 12

        # ---------- Ut = g * U (triangle) ----------
        for k in range(DOF):
            tt(dap(o["Ut"] + 42 * k, [(6, k + 1), (1, 6)]),
               cap(co["g"] + 6 * k, [(0, k + 1), (1, 6)]),
               dap(rows0 + 54 * k + 12, [(6, k + 1), (1, 6)]))

        # ---------- V rolls (w, v parts of Vs; wr1/vr1 and wr2/vr2 are
        # adjacent 21-ch blocks, so each roll-piece copies both at once) ----
        # roll1: dst[0:2] = src[1:3], dst[2] = src[0]   (w and v parts)
        cp(dap(o["wr1"], [(21, 2), (3, 7), (1, 2)]),
           dap(rows0 + 1, [(3, 2), (54, 7), (1, 2)]))
        cp(dap(o["wr1"] + 2, [(21, 2), (3, 7)]),
           dap(rows0, [(3, 2), (54, 7)]))
        # roll2: dst[0] = src[2], dst[1:3] = src[0:2]
        cp(dap(o["wr2"], [(21, 2), (3, 7)]),
           dap(rows0 + 2, [(3, 2), (54, 7)]))
        cp(dap(o["wr2"] + 1, [(21, 2), (3, 7), (1, 2)]),
           dap(rows0, [(3, 2), (54, 7), (1, 2)]))

        # gwv1 = [gRr1|gm3] * [wr1|vr1] ; gwv2 = [gRr2|gm3] * [wr2|vr2]
        tt(dap(o["gwv1"], [(1, 42)]), cap(co["gA1"], [(1, 42)]),
           dap(o["wr1"], [(1, 42)]))
        tt(dap(o["gwv2"], [(1, 42)]), cap(co["gA2"], [(1, 42)]),
           dap(o["wr2"], [(1, 42)]))

        # ---------- Y = G Vd + (wxyw + vxyv ; wxyv) ----------
        # 4-combo cross products: [w,w,v,v] x [yw,yv,yw,yv] (one waste slot)
        tt(dap(o["Y"], [(6, 7), (1, 6)]), cap(co["g"], [(6, 7), (1, 6)]),
           dap(rows0 + 6, [(54, 7), (1, 6)]))
        tt(dap(o["tcp"], [(42, 2), (21, 2), (1, 21)]),
           dap(o["wr1"], [(21, 2), (0, 2), (1, 21)]),
           dap(o["gwv2"], [(0, 2), (21, 2), (1, 21)]))
        tt(dap(o["tcn"], [(42, 2), (21, 2), (1, 21)]),
           dap(o["wr2"], [(21, 2), (0, 2), (1, 21)]),
           dap(o["gwv1"], [(0, 2), (21, 2), (1, 21)]))
        tt(dap(o["tcs"], [(1, 84)]), dap(o["tcp"], [(1, 84)]),
           dap(o["tcn"], [(1, 84)]), AO.subtract)
        # Y_w += t[0] (wxyw) + t[3] (vxyv); Y_v += t[1] (wxyv)
        tt(dap(o["Y"], [(6, 7), (1, 3)]), dap(o["Y"], [(6, 7), (1, 3)]),
           dap(o["tcs"], [(3, 7), (1, 3)]), AO.add)
        tt(dap(o["Y"], [(6, 7), (1, 3)]), dap(o["Y"], [(6, 7), (1, 3)]),
           dap(o["tcs"] + 63, [(3, 7), (1, 3)]), AO.add)
        tt(dap(o["Y"] + 3, [(6, 7), (1, 3)]), dap(o["Y"] + 3, [(6, 7), (1, 3)]),
           dap(o["tcs"] + 21, [(3, 7), (1, 3)]), AO.add)

        # ---------- bias_i = sum_{k>=i} U_{k,i} . Y_k ----------
        for i in range(DOF):
            nk = DOF - i
            tt(dap(o["tbias"], [(6, nk), (1, 6)]),
               dap(rows0 + 54 * i + 12 + 6 * i, [(54, nk), (1, 6)]),
               dap(o["Y"] + 6 * i, [(6, nk), (1, 6)]))
            red(dap(o["bias"] + i, []),
                dap(o["tbias"], [(6, nk), (1, 6)], bpos=0), axis="X")
        # rhs (= qacc block) = tau - bias
        tt(dap(o["qacc"] + 7 * stage, [(1, 7)]), dap(o["tau"], [(1, 7)]),
           dap(o["bias"], [(1, 7)]), AO.subtract)

        # ---------- Gram mass matrix (lower triangle rows) ----------
        for j in range(DOF):
            nk = DOF - j
            ni = j + 1
            tt(dap(o["tgram"], [(6 * nk, ni), (6, nk), (1, 6)]),
               dap(rows0 + 54 * j + 12, [(6, ni), (54, nk), (1, 6)]),
               dap(o["Ut"] + 42 * j + 6 * j, [(0, ni), (42, nk), (1, 6)]))
            red(dap(o["Mg"] + 7 * j, [(1, ni)]),
                dap(o["tgram"], [(6 * nk, ni), (1, 6 * nk)], bpos=1))

        # ---------- LDL^T ----------
        rhs = o["qacc"] + 7 * stage
        for j in range(DOF):
            V.reciprocal(dap(o["rinv"] + j, []), dap(o["Mg"] + 8 * j, []))
            nr = DOF - 1 - j
            if nr == 0:
                break
            tt(dap(o["Lg"] + 7 * (j + 1) + j, [(7, nr)]),
               dap(o["Mg"] + 7 * (j + 1) + j, [(7, nr)]),
               dap(o["rinv"] + j, [(0, nr)]))
            tt(dap(o["tso"], [(nr, nr), (1, nr)]),
               dap(o["Lg"] + 7 * (j + 1) + j, [(7, nr), (0, nr)]),
               dap(o["Mg"] + 7 * (j + 1) + j, [(0, nr), (7, nr)]))
            tt(dap(o["Mg"] + 8 * (j + 1), [(7, nr), (1, nr)]),
               dap(o["Mg"] + 8 * (j + 1), [(7, nr), (1, nr)]),
               dap(o["tso"], [(nr, nr), (1, nr)]), AO.subtract)
        for j in range(DOF - 1):
            nr = DOF - 1 - j
            tt(dap(o["tss"], [(1, nr)]),
               dap(o["Lg"] + 7 * (j + 1) + j, [(7, nr)]),
               dap(rhs + j, [(0, nr)]))
            tt(dap(rhs + j + 1, [(1, nr)]), dap(rhs + j + 1, [(1, nr)]),
               dap(o["tss"], [(1, nr)]), AO.subtract)
        tt(dap(rhs, [(1, 7)]), dap(rhs, [(1, 7)]), dap(o["rinv"], [(1, 7)]))
        for j in range(DOF - 1, 0, -1):
            tt(dap(o["tss"], [(1, j)]), dap(o["Lg"] + 7 * j, [(1, j)]),
               dap(rhs + j, [(0, j)]))
            tt(dap(rhs, [(1, j)]), dap(rhs, [(1, j)]),
               dap(o["tss"], [(1, j)]), AO.subtract)


    for _s in range(4):
        if _s > 0:
            prep_qs(_s)
            prep_dqs(_s)
        emit_build(_s)
        emit_dyn(_s)

    # ---------- RK4 combine ----------
    # qn = q0 + H/6 * sum w_s*dqs_s ; dqn = dq0 + H/6 * sum w_s*qacc_s
    tt(dap(o["tw"], [(7, 4), (1, 7)]), cap(co["w4"], [(1, 4), (0, 7)]),
       dap(o["dq0"], [(7, 4), (1, 7)]))
    red(dap(o["tk7"], [(1, 7)]), dap(o["tw"], [(1, 7), (7, 4)], bpos=1))
    V.scalar_tensor_tensor(dap(o["qn"], [(1, 7)]), dap(o["tk7"], [(1, 7)]),
        H / 6.0, dap(o["q0"], [(1, 7)]), AO.mult, AO.add)
    tt(dap(o["tw"], [(7, 4), (1, 7)]), cap(co["w4"], [(1, 4), (0, 7)]),
       dap(o["qacc"], [(7, 4), (1, 7)]))
    red(dap(o["tk7"], [(1, 7)]), dap(o["tw"], [(1, 7), (7, 4)], bpos=1))
    V.scalar_tensor_tensor(dap(o["dqn"], [(1, 7)]), dap(o["tk7"], [(1, 7)]),
        H / 6.0, dap(o["dq0"], [(1, 7)]), AO.mult, AO.add)

    # wrap q -> outch[0:7]
    wrap_to(o["outch"], o["qn"])
    # clip dq -> outch[7:14]
    ts(dap(o["outch"] + 7, [(1, 7)]), dap(o["dqn"], [(1, 7)]),
       -MAX_VELOCITY, MAX_VELOCITY, AO.max, AO.min)

    # ---------- FK ----------
    qw = o["outch"]
    sincos(o["s2"], o["c2"], qw)
    ts(dap(o["oc2"], [(1, 7)]), dap(o["c2"], [(1, 7)]), -1.0, 1.0,
       AO.mult, AO.add)

    tt(dap(o["tb1"], [(9, 7), (1, 9)]), dap(o["s2"], [(1, 7), (0, 9)]),
       cap(co["F1"], [(9, 7), (1, 9)]))
    tt(dap(o["RB"], [(1, 63)]), cap(co["F0"], [(1, 63)]),
       dap(o["tb1"], [(1, 63)]), AO.add)
    tt(dap(o["tb1"], [(9, 7), (1, 9)]), dap(o["oc2"], [(1, 7), (0, 9)]),
       cap(co["F2"], [(9, 7), (1, 9)]))
    tt(dap(o["RB"], [(1, 63)]), dap(o["RB"], [(1, 63)]),
       dap(o["tb1"], [(1, 63)]), AO.add)

    tt(dap(o["tb1"], [(3, 7), (1, 3)]), dap(qw, [(1, 7), (0, 3)]),
       cap(co["f1"], [(3, 7), (1, 3)]))
    tt(dap(o["pB"], [(1, 21)]), cap(co["f0"], [(1, 21)]),
       dap(o["tb1"], [(1, 21)]), AO.add)
    for src, cname in ((o["s2"], "f2"), (o["oc2"], "f3")):
        tt(dap(o["tb1"], [(3, 7), (1, 3)]), dap(src, [(1, 7), (0, 3)]),
           cap(co[cname], [(3, 7), (1, 3)]))
        tt(dap(o["pB"], [(1, 21)]), dap(o["pB"], [(1, 21)]),
           dap(o["tb1"], [(1, 21)]), AO.add)

    cp(dap(o["rcur"], [(1, 6)]), cap(co["r01i"], [(1, 6)]))
    V.memset(dap(o["outch"] + 14, [(1, 2)]), 0.0)
    cur, nxt = o["rcur"], o["rnxt"]
    for i in range(DOF):
        # p01 += rows . pB_i
        tt(dap(o["tfd"], [(3, 2), (1, 3)]), dap(cur, [(3, 2), (1, 3)]),
           dap(o["pB"] + 3 * i, [(0, 2), (1, 3)]))
        red(dap(o["tpd"], [(1, 2)]), dap(o["tfd"], [(3, 2), (1, 3)], bpos=1))
        tt(dap(o["outch"] + 14, [(1, 2)]), dap(o["outch"] + 14, [(1, 2)]),
           dap(o["tpd"], [(1, 2)]), AO.add)
        # rows = rows @ RB_i  (per row; P[c][c'] = r[c]*RB[c][c'])
        for row in range(2):
            tt(dap(o["tfr"] + 9 * row, [(3, 3), (1, 3)]),
               dap(cur + 3 * row, [(1, 3), (0, 3)]),
               dap(o["RB"] + 9 * i, [(3, 3), (1, 3)]))
            red(dap(nxt + 3 * row, [(1, 3)]),
                dap(o["tfr"] + 9 * row, [(1, 3), (3, 3)], bpos=1))
        cur, nxt = nxt, cur
    tt(dap(o["tfd"], [(3, 2), (1, 3)]), dap(cur, [(3, 2), (1, 3)]),
       cap(co["pe"], [(0, 2), (1, 3)]))
    red(dap(o["tpd"], [(1, 2)]), dap(o["tfd"], [(3, 2), (1, 3)], bpos=1))
    tt(dap(o["outch"] + 14, [(1, 2)]), dap(o["outch"] + 14, [(1, 2)]),
       dap(o["tpd"], [(1, 2)]), AO.add)

    # ---------- interleave + store ----------
    cp(bass.AP(S.tensor, S.offset + o["stag_out"] * T,
               [list(S.ap[0])] + [[1, 16], [16, T]]),
       dap(o["outch"], [(1, 16)], bpos=1))
    nc.sync.dma_start(out_d[:].rearrange("(p t) c -> p (t c)", p=P),
                      S[:, o["stag_out"] * T:(o["stag_out"] + 16) * T])


def _split_waits(nc, maxw=1):
    """This walrus build accepts at most one embedded sync-wait per
    instruction; hoist extras into standalone NoOps on the same engine."""
    from concourse import mybir

    for fn in nc.m.functions:
        for blk in fn.blocks:
            new = []
            for inst in blk.instructions:
                si = getattr(inst, "sync_info", None)
                if si is not None and si.on_wait and len(si.on_wait) > maxw:
                    waits = list(si.on_wait)
                    for k, w in enumerate(waits[maxw:]):
                        new.append(mybir.InstNoOp(
                            name=f"{inst.name}-w{k}",
                            sync_info=mybir.SyncInfo(on_wait=[w], on_update=[]),
                            engine=inst.engine,
                        ))
                    si.on_wait = waits[:maxw]
                new.append(inst)
            blk.instructions = new


# ============================ host entry ============================
_BUILT = None


def _build():
    global _BUILT
    if _BUILT is not None:
        return _BUILT
    import concourse.bass as bass
    import concourse.tile as tile
    from concourse import mybir

    f32 = mybir.dt.float32
    nc = bass.Bass()
    state_d = nc.declare_dram_parameter("state", [PB, 14], f32, isOutput=False)
    action_d = nc.declare_dram_parameter("action", [PB, 7], f32,
                                         isOutput=False)
    consts_d = nc.declare_dram_parameter("consts", [P, NCOL], f32,
                                         isOutput=False)
    out_d = nc.declare_dram_parameter("out", [PB, 16], f32, isOutput=True)
    with tile.TileContext(nc) as tc:
        with tc.tile_pool(name="main", bufs=1) as pool:
            emit(nc, tc, pool, state_d, action_d, consts_d, out_d)
    _split_waits(nc)
    _BUILT = nc
    return nc


def run_cores(state, action, consts, trace=False):
    from concourse.bass_utils import run_bass_kernel_spmd

    nc = _build()
    state = np.ascontiguousarray(np.asarray(state, np.float32))
    action = np.ascontiguousarray(np.asarray(action, np.float32))
    in_maps = []
    for c in range(NCORES):
        sl = slice(c * PB, (c + 1) * PB)
        in_maps.append({"state": state[sl], "action": action[sl],
                        "consts": consts})
    res = run_bass_kernel_spmd(nc, in_maps, core_ids=list(range(NCORES)),
                               trace=trace)
    out = np.concatenate([res.results[i]["out"] for i in range(NCORES)], 0)
    return out, res


def kernel(state, action, _M, _A, _G, gravity):
    consts = make_consts(_M, _A, _G, gravity)
    out, _ = run_cores(state, action, consts)
    return out[:, :14].copy(), out[:, 14:16].copy()
